# revision 17
# baseline (speedup 1.0000x reference)
"""Expert-parallel MoE (top-2, E=8) for one Trainium2 chip (8 NeuronCores).

Contract: kernel(**inputs) takes the FULL unsharded inputs
  x  [4, 2048, 1024] f32,  Wr [1024, 8] f32,
  W1 [8, 1024, 2730] f32,  W2 [8, 2730, 1024] f32,  W3 [8, 1024, 2730] f32
and returns the FULL output [4, 2048, 1024] f32.

Sharding strategy (half-expert pairing, 2 cores per expert pair):
  - The tiny router (softmax + top-2 over 8 experts) runs on host in fp32.
  - Experts are paired hot-with-cold: the 4 largest loads are "section 1",
    the 4 smallest "section 2".  The pair's two cores BOTH process the
    union of both experts' tokens, but each core owns one H-half
    (1365 -> padded 1408 = 11x128) of both experts' W1/W3/W2.  The two
    partial y outputs are summed on host during the combine, so the
    per-core critical load drops from max_e C_e to (C_i + C_j)/2-ish.
  - One SPMD program serves all 8 cores: token sections are padded to the
    uniform sizes S1 = max first-counts, S2 = max second-counts, and each
    block of columns uses either weight-set A (section 1) or B (section 2).
  - Per block the core computes the half-H SwiGLU FFN out of SBUF-resident
    weights:  Y^T = W2h^T @ (silu(W1h^T @ X^T) * (W3h^T @ X^T))
    fp16 operands, fp32 PSUM accumulation, fp32 output.
  - Block plan avoids tiny tail blocks (tail < ~227 columns is
    LDWEIGHTS-bound and wastes PE issue slots): remainders > 512 are split
    into two near-equal blocks.
  - Host combine: out[tok] = sum_k gate[tok,k] * (Y_lo + Y_hi)[pos_k].
"""

import copy
import json
import math
from contextlib import ExitStack

import numpy as np

# ---------------------------------------------------------------------------
# Walrus workaround: the neuronxcc walrus in this environment supports only
# ONE sync wait per instruction, while the Tile framework emits a final Drain
# carrying several.  Rewrite the serialized BIR: hoist extra waits into
# wait-only EventSemaphore instructions placed immediately before, on the
# same engine (the sequencer blocks on them in program order, so the
# semantics are unchanged).
# ---------------------------------------------------------------------------


def _split_multiwait_bir(bir_json):
    d = json.loads(bir_json)
    changed = False
    multi_update = []
    for fn in d.get("functions", []):

        def walk(block):
            nonlocal changed
            il = block.get("instructions")
            if il:
                new = []
                blk_changed = False
                for i in il:
                    si = i.get("sync_info") or {}
                    ws = si.get("on_wait") or []
                    if len(ws) > 1:
                        for j, w in enumerate(ws[:-1]):
                            new.append(
                                {
                                    "debug": i.get("debug"),
                                    "engine": i["engine"],
                                    "ins": [],
                                    "outs": [],
                                    "name": f"{i['name']}_xw{j}",
                                    "opcode": "EventSemaphore",
                                    "sync_info": {"on_update": [], "on_wait": [w]},
                                }
                            )
                        i = copy.deepcopy(i)
                        i["sync_info"]["on_wait"] = [ws[-1]]
                        blk_changed = True
                    us = (i.get("sync_info") or {}).get("on_update") or []
                    if len(us) > 1:
                        multi_update.append((i.get("name"), i.get("opcode")))
                    new.append(i)
                if blk_changed:
                    block["instructions"] = new
                    changed = True
            for b in block.get("blocks", []) or []:
                walk(b)

        walk(fn)

        # Trim the post-drain barrier/sem-clear tail of the TileContext end
        # block (~5-10 us of EVSEM butterfly).  The Drain already guarantees
        # all output DMAs completed; sems are re-initialized by the preamble
        # on the next execution (verified by back-to-back runs).
        def trim(block):
            nonlocal changed
            il = block.get("instructions")
            if il and block.get("name", "").endswith("_end"):
                last_drain = None
                for idx, i in enumerate(il):
                    if i.get("opcode") == "Drain" and i.get("engine") == "SP":
                        last_drain = idx
                        break
                if last_drain is not None and last_drain + 1 < len(il):
                    block["instructions"] = il[: last_drain + 1]
                    changed = True
            for b in block.get("blocks", []) or []:
                trim(b)

        trim(fn)
    if multi_update:
        raise RuntimeError(f"multi-update instructions unsupported: {multi_update[:5]}")
    if not changed:
        return bir_json
    return json.dumps(d).encode()


_patched = False


def _install_bir_patch():
    global _patched
    if _patched:
        return
    import concourse.bass2jax as b2j

    orig = b2j.compile_bir_kernel

    def patched(bir_json, tmpdir, neff_name="file.neff"):
        return orig(_split_multiwait_bir(bir_json), tmpdir, neff_name)

    b2j.compile_bir_kernel = patched
    _patched = True


_install_bir_patch()

import concourse.bass as bass
import concourse.mybir as mybir
import concourse.tile as tile
from concourse.bass_utils import run_bass_kernel_spmd

D = 1024
E = 8
TOP_K = 2
H = 2730
HH = 1365  # H // 2, split point of the two H-halves
HHP = 1408  # H-half padded to 11 * 128
DT = mybir.dt.float16
NP_DT = np.float16
D_TILES = D // 128  # 8
H_TILES = HHP // 128  # 11


def _plan_blocks(C, small_first=False):
    # Avoid tails < ~227 columns: below that the per-block PE cost is
    # LDWEIGHTS/dispatch-bound, not streaming-bound, so tiny tails waste
    # ~10s of us.  Remainders in (512, 768) are split into two halves.
    blocks = []
    rem = C
    while rem >= 768:
        blocks.append(512)
        rem -= 512
    if rem > 512:
        blocks.append((rem + 1) // 2)
        blocks.append(rem - (rem + 1) // 2)
    elif rem:
        blocks.append(rem)
    if small_first:
        # Smallest block first: its x_pre DMA lands sooner, so the real
        # MM stream starts earlier off a shorter warmup.
        blocks.sort()
    return blocks


def _build_nc(S1, S2):
    T = S1 + S2
    # (weight-set, column offset, block width); section 1 = set A.
    sched = [(0, off, TB) for off, TB in _offsets(_plan_blocks(S1))]
    sched += [(1, S1 + off, TB) for off, TB in _offsets(_plan_blocks(S2))]
    nc = bass.Bass()
    f32 = mybir.dt.float32

    xt = nc.dram_tensor("xt", [D, T], DT, kind="ExternalInput")
    wts = []
    for s in "ab":
        wts.append(
            (
                nc.dram_tensor(f"w1{s}", [D, HHP], DT, kind="ExternalInput"),
                nc.dram_tensor(f"w3{s}", [D, HHP], DT, kind="ExternalInput"),
                nc.dram_tensor(f"w2{s}", [HHP, D], DT, kind="ExternalInput"),
            )
        )
    yt = nc.dram_tensor("yt", [D, T], f32, kind="ExternalOutput")

    with tile.TileContext(nc) as tc, ExitStack() as ctx:
        wpool = ctx.enter_context(tc.tile_pool(name="w", bufs=1))
        xpool = ctx.enter_context(tc.tile_pool(name="x", bufs=3))
        hpool = ctx.enter_context(tc.tile_pool(name="h", bufs=1))
        spool = ctx.enter_context(tc.tile_pool(name="s", bufs=3))
        ypool = ctx.enter_context(tc.tile_pool(name="y", bufs=5))
        psA = ctx.enter_context(tc.tile_pool(name="psA", bufs=4, space="PSUM"))
        psY = ctx.enter_context(tc.tile_pool(name="psY", bufs=4, space="PSUM"))

        # DRAM views with the 128-partition dim split out so one dma_start
        # covers all row-tiles of a column chunk (each dma_start costs
        # ~650 ns of serial sequencer dispatch: fewer + bigger wins).
        xt_v = xt.rearrange("(d p) c -> p d c", p=128)

        # Dependency-free warmup matmuls: keep the PE busy from t=0 so the
        # HAM clock gate opens (1.2 -> 2.4 GHz) before the first real
        # matmul group's weights arrive over DMA.
        warm = ypool.tile([128, 256], DT, tag="warm")
        wps = psA.tile([128, 512], f32, tag="psA")
        # ~16 cold MMs (3.4us) flip HAM to 2.4 GHz; the rest bridge until
        # the first x block + w1a chunk land (~15us).  Too few and the PE
        # idles cold into the real stream; too many delays it.
        for _ in range(48):
            nc.tensor.matmul(
                wps[:, :256], lhsT=warm[:, :128], rhs=warm[:, :256], start=True, stop=True
            )
        # warm is read uninitialized on purpose: the products land in a PSUM
        # tile that is never consumed, and skipping the memset removes the
        # DVE-preamble dependency so the PE warms from t~=3us.
        nc.vector.memset(warm[:], 0.0)

        def load_x(off, TB, eng):
            x_sb = xpool.tile([128, D_TILES, TB], DT, tag="x")
            eng.dma_start(x_sb[:], xt_v[:, :, off : off + TB])
            return x_sb

        # First token block loads before the weight stream (split across the
        # two HWDGE rings) so the first matmul group unblocks early.
        TB0 = sched[0][2]
        x_pre = xpool.tile([128, D_TILES, TB0], DT, tag="x")
        nc.gpsimd.dma_start(x_pre[:, 0:3], xt_v[:, 0:3, 0:TB0])
        nc.scalar.dma_start(x_pre[:, 3:6], xt_v[:, 3:6, 0:TB0])
        nc.sync.dma_start(x_pre[:, 6:8], xt_v[:, 6:8, 0:TB0])

        # SBUF-resident weights, streamed in phase-A consumption order:
        # set A fully (its phase-A stream, then its W2), then set B.
        w_sb = []
        for si, (w1, w3, w2) in enumerate(wts):
            w1_v = w1.rearrange("(d p) h -> p d h", p=128)
            w3_v = w3.rearrange("(d p) h -> p d h", p=128)
            w2_v = w2.rearrange("(h p) d -> p h d", p=128)
            w1_sb = wpool.tile([128, D_TILES, HHP], DT, tag=f"w1{si}")
            w3_sb = wpool.tile([128, D_TILES, HHP], DT, tag=f"w3{si}")
            w2_sb = wpool.tile([128, H_TILES, D], DT, tag=f"w2{si}")
            hc_off = 0
            chunks = [128, 128, 256, 256, 640] if si == 0 else [704, 704]
            for hc in chunks:
                sl = slice(hc_off, hc_off + hc)
                nc.sync.dma_start(w1_sb[:, :, sl], w1_v[:, :, sl])
                nc.sync.dma_start(w3_sb[:, :, sl], w3_v[:, :, sl])
                hc_off += hc
            assert hc_off == HHP
            for h_i in range(0, H_TILES, 6):
                nh = min(6, H_TILES - h_i)
                nc.sync.dma_start(w2_sb[:, h_i : h_i + nh], w2_v[:, h_i : h_i + nh, :])
            w_sb.append((w1_sb, w3_sb, w2_sb))

        for bi, (si, off, TB) in enumerate(sched):
            w1_sb, w3_sb, w2_sb = w_sb[si]
            x_sb = x_pre if bi == 0 else load_x(off, TB, nc.gpsimd)

            # Phase A: H^T[:, block] = silu(W1h^T X^T) * (W3h^T X^T), fp16.
            h_sb = hpool.tile([128, H_TILES, TB], DT, tag="h")
            for h_i in range(H_TILES):
                ps1 = psA.tile([128, TB], f32, tag="psA")
                for d_i in range(D_TILES):
                    nc.tensor.matmul(
                        ps1,
                        lhsT=w1_sb[:, d_i, h_i * 128 : (h_i + 1) * 128],
                        rhs=x_sb[:, d_i],
                        start=(d_i == 0),
                        stop=(d_i == D_TILES - 1),
                    )
                ps3 = psA.tile([128, TB], f32, tag="psA")
                for d_i in range(D_TILES):
                    nc.tensor.matmul(
                        ps3,
                        lhsT=w3_sb[:, d_i, h_i * 128 : (h_i + 1) * 128],
                        rhs=x_sb[:, d_i],
                        start=(d_i == 0),
                        stop=(d_i == D_TILES - 1),
                    )
                sil = spool.tile([128, TB], f32, tag="sil")
                nc.scalar.activation(sil, ps1, mybir.ActivationFunctionType.Silu)
                nc.vector.tensor_mul(h_sb[:, h_i], sil, ps3)

            # Phase B: Y^T[:, block] = W2h^T @ H^T.
            for m_i in range(D_TILES):
                psy = psY.tile([128, TB], f32, tag="psY")
                for h_i in range(H_TILES):
                    nc.tensor.matmul(
                        psy,
                        lhsT=w2_sb[:, h_i, m_i * 128 : (m_i + 1) * 128],
                        rhs=h_sb[:, h_i],
                        start=(h_i == 0),
                        stop=(h_i == H_TILES - 1),
                    )
                y_sb = ypool.tile([128, TB], f32, tag="y")
                nc.vector.tensor_copy(y_sb, psy)
                # y goes out on the scalar ring: the sync ring carries the
                # 17.3MB weight stream and y DMAs queue behind it otherwise.
                nc.scalar.dma_start(
                    yt[m_i * 128 : (m_i + 1) * 128, off : off + TB], y_sb
                )

    return nc


def _offsets(blocks):
    out, off = [], 0
    for b in blocks:
        out.append((off, b))
        off += b
    return out


def _route(flat, Wr):
    N = flat.shape[0]
    logits = flat @ Wr
    m = logits.max(-1, keepdims=True)
    p = np.exp(logits - m)
    p /= p.sum(-1, keepdims=True)
    topi = np.argsort(-p, axis=-1)[:, :TOP_K]
    topv = np.take_along_axis(p, topi, -1)

    assign_tok = np.tile(np.arange(N), TOP_K)
    assign_exp = topi.T.ravel()
    order = np.argsort(assign_exp, kind="stable")
    counts = np.bincount(assign_exp, minlength=E)
    starts = np.zeros(E + 1, np.int64)
    starts[1:] = np.cumsum(counts)
    pos = np.empty(N * TOP_K, np.int64)
    pos[order] = np.arange(N * TOP_K) - starts[assign_exp[order]]
    return topv, assign_tok, assign_exp, order, counts, starts, pos


_NC_CACHE = {}


def _half_weights(W1e, W2e, W3e, lo):
    sl = slice(0, HH) if lo else slice(HH, H)
    n = HH if lo else H - HH
    w1 = np.zeros((D, HHP), NP_DT)
    w1[:, :n] = W1e[:, sl].astype(NP_DT)
    w3 = np.zeros((D, HHP), NP_DT)
    w3[:, :n] = W3e[:, sl].astype(NP_DT)
    w2 = np.zeros((HHP, D), NP_DT)
    w2[:n, :] = W2e[sl, :].astype(NP_DT)
    return w1, w3, w2


def kernel(x, Wr, W1, W2, W3, _trace=False, _result=None):
    x = np.asarray(x)
    Wr = np.asarray(Wr, dtype=np.float32)
    W1 = np.asarray(W1)
    W2 = np.asarray(W2)
    W3 = np.asarray(W3)
    Bx, Tx, Dx = x.shape
    N = Bx * Tx
    flat = np.ascontiguousarray(x.reshape(N, Dx).astype(np.float32))

    topv, assign_tok, assign_exp, order, counts, starts, pos = _route(flat, Wr)

    # Pair the 4 most-loaded experts (section 1) with the 4 least-loaded
    # (section 2).  One SPMD program: sections padded to S1/S2 on all cores.
    by_load = np.argsort(-counts, kind="stable")
    firsts, seconds = by_load[:4], by_load[4:]
    S1 = max(128, int(counts[firsts].max()))
    S2 = max(128, int(counts[seconds].max()))
    T = S1 + S2

    flat16 = flat.astype(NP_DT)
    in_maps = [None] * E
    core_of = {}  # expert -> (core_lo, core_hi, col_offset)
    for k in range(4):
        ei, ej = int(firsts[k]), int(seconds[k])
        xte = np.zeros((D, T), NP_DT)
        idx_i = assign_tok[order[starts[ei] : starts[ei + 1]]]
        idx_j = assign_tok[order[starts[ej] : starts[ej + 1]]]
        xte[:, : counts[ei]] = flat16[idx_i].T
        xte[:, S1 : S1 + counts[ej]] = flat16[idx_j].T
        core_of[ei] = (2 * k, 2 * k + 1, 0)
        core_of[ej] = (2 * k, 2 * k + 1, S1)
        for half in (0, 1):
            w1a, w3a, w2a = _half_weights(W1[ei], W2[ei], W3[ei], half == 0)
            w1b, w3b, w2b = _half_weights(W1[ej], W2[ej], W3[ej], half == 0)
            in_maps[2 * k + half] = {
                "xt": xte,
                "w1a": w1a, "w3a": w3a, "w2a": w2a,
                "w1b": w1b, "w3b": w3b, "w2b": w2b,
            }

    if (S1, S2) not in _NC_CACHE:
        _NC_CACHE[(S1, S2)] = _build_nc(S1, S2)
    nc = _NC_CACHE[(S1, S2)]

    res = run_bass_kernel_spmd(nc, in_maps, list(range(E)), trace=_trace)
    if _result is not None:
        _result.append(res)

    # Combine: Y_e = lo-half partial + hi-half partial, then gate-scatter.
    Y = np.empty((E, D, S1), np.float32)
    for e_i in range(E):
        lo, hi, off = core_of[e_i]
        c = counts[e_i]
        Y[e_i, :, :c] = (
            res.results[lo]["yt"][:, off : off + c]
            + res.results[hi]["yt"][:, off : off + c]
        )
    out = np.zeros((N, D), np.float32)
    for k in range(TOP_K):
        sl = slice(k * N, (k + 1) * N)
        out += topv[:, k, None] * Y[assign_exp[sl], :, pos[sl]]
    return out.reshape(Bx, Tx, Dx).astype(x.dtype)

